# revision 75
# baseline (speedup 1.0000x reference)
"""Causal self-attention (B=2, S=2048, D=1024, H=16) on 8 Trainium2 cores.

Sharding: core c = (batch b = c // 4, head group hg = c % 4, 4 heads each).

v3: host pre-transposes x (no on-chip x transposes) and ships it twice:
  - xT8  [128, 4, 2, 2048] fp8  (DoubleRow-packed: d = 256c + 128j + i)
  - xTb  [128, 8, 2048]    bf16 (plain d-chunks, for the v projection)
q/k projections run fp8 DoubleRow (contract 256/step, 4 steps); the v
projection stays bf16 (fp8 there breaks the 2e-2 error budget). Scores run
fp8 DoubleRow over the packed q8/k8 layout [32*h + p, pack, s]. The causal
mask for diagonal blocks is fused into the scores PSUM accumulation as an
extra matmul (identity stationary x triangular -1000 moving), so exp
output feeds attn@v directly. attn@v uses exp output as STATIONARY
([128 kpos, 128 q]) with rhs = [v | ones]; the ones column yields the
softmax denominator.

Schedule: one flat software-pipelined stream over (block g, head pair,
key tile j). exp on ACT paces it; the next step's scores matmuls are
emitted immediately behind each exp so they never queue behind the
step's own attn@v work on the in-order PE. Filler work (q/k of g+1,
v chains of g with per-step deadlines, c_proj of g-1) is split into
small atoms with estimated PE cost and emitted by a slack accountant.
Divisions run inline per q-tile as each av chain stops; the last
block's c_proj transposes emit at those points with its matmuls/stores
batched after, so only ~one q-tile's chain trails the final exp. PE
warmup transposes at boot hold the p-state while DMAs land. Host sums
the 4 per-batch partials (+ b_proj) at the end.
"""

import os
import sys

for _p in ("/opt/trn_rl_repo", os.path.expanduser("~/.axon_site/_ro/trn_rl_repo")):
    if os.path.isdir(_p) and _p not in sys.path:
        sys.path.insert(0, _p)
        break

import numpy as np

import concourse.bass as bass
import concourse.mybir as mybir
import concourse.tile as tile
from concourse import bacc
from concourse.masks import make_identity

B, S, D = 2, 2048, 1024
H, HD = 16, 64
NCORES = 8
GB = 4            # cores per batch
NH = H // GB      # heads per core = 4
CD = NH * HD      # q/k/v columns per core = 256
P = 128
ST = S // P       # 16 seq tiles
DC = D // P       # 8 contraction chunks of D
DRC = D // 256    # 4 DoubleRow contraction steps
QB = 512          # q block width
NQB = S // QB     # 4
CC = CD // P      # 2 col chunks

F32 = mybir.dt.float32
BF = mybir.dt.bfloat16
F8 = mybir.dt.float8e4
DR = mybir.MatmulPerfMode.DoubleRow

PE_NS = 1.0 / 2.4          # ns per PE cycle (warm)
ACT_NS = 1.0 / 1.2         # ns per ACT cycle


def build_nc(reps: int = 1):
    nc = bacc.Bacc("TRN2", target_bir_lowering=False, debug=False,
                   num_devices=NCORES)

    xt8_d = nc.dram_tensor("xt8", [P, DRC, 2, S], F8, kind="ExternalInput").ap()
    xtb_d = nc.dram_tensor("xtb", [P, DC, S], BF, kind="ExternalInput").ap()
    wq_d = nc.dram_tensor("wq", [P, DRC, 2, CD], F8, kind="ExternalInput").ap()
    wk_d = nc.dram_tensor("wk", [P, DRC, 2, CD], F8, kind="ExternalInput").ap()
    wv_d = nc.dram_tensor("wv", [P, DC, CD], BF, kind="ExternalInput").ap()
    bq_d = nc.dram_tensor("bq", [CD], F32, kind="ExternalInput").ap()
    bk_d = nc.dram_tensor("bk", [CD], F32, kind="ExternalInput").ap()
    bv_d = nc.dram_tensor("bv", [CD], F32, kind="ExternalInput").ap()
    wo_d = nc.dram_tensor("wo", [P, CC, D], BF, kind="ExternalInput").ap()
    # partials are summed on the host across 4 cores: bf16 storage halves
    # the store DMA volume for ~1e-3 relative noise on ~0.2-magnitude values
    out_d = nc.dram_tensor("out", [S, D], BF, kind="ExternalOutput").ap()

    out_v = out_d.rearrange("(o p) d -> p o d", p=P)    # [128, 16, 1024]

    with tile.TileContext(nc) as tc:
        with (
            tc.tile_pool(name="const", bufs=1) as const,
            tc.tile_pool(name="wpool", bufs=1) as wpool,
            tc.tile_pool(name="persist", bufs=1) as persist,
            tc.tile_pool(name="expp", bufs=3) as expp,
            tc.tile_pool(name="hpool", bufs=2) as hpool,
            tc.tile_pool(name="outp", bufs=2) as outp,
            tc.tile_pool(name="ps", bufs=2, space="PSUM") as ps,
        ):
            # ---- persistent activations (declared first for clarity) ----
            xt8_sb = persist.tile([P, DRC, 2, S], F8)
            xtb_sb = persist.tile([P, DC, S], BF)
            q8 = persist.tile([P, CC, S], F8)
            k8 = persist.tile([P, CC, S], F8)
            v1 = persist.tile([P, ST, NH, HD + 1], BF)

            # ---- weights ----
            wq_sb = wpool.tile([P, DRC, 2, CD], F8)
            wk_sb = wpool.tile([P, DRC, 2, CD], F8)
            wv_sb = wpool.tile([P, DC, CD], BF)
            wo_sb = wpool.tile([P, CC, D], BF)

            def load_x8(g):
                sl = slice(g * QB, (g + 1) * QB)
                nc.sync.dma_start(xt8_sb[:, :, :, sl], xt8_d[:, :, :, sl])

            def load_xb(g):
                sl = slice(g * QB, (g + 1) * QB)
                nc.sync.dma_start(xtb_sb[:, :, sl], xtb_d[:, :, sl])

            # priority-ordered prefetch: the critical chain to the first exp
            # is xt8(g0) -> wq -> wk -> bq/bk; everything else streams behind.
            bq_sb = const.tile([P, CC], F32)
            bk_sb = const.tile([P, CC], F32)
            bv_row = const.tile([1, CD], F32)
            load_x8(0)
            nc.sync.dma_start(wq_sb[:], wq_d)
            nc.sync.dma_start(wk_sb[:], wk_d)
            nc.sync.dma_start(bq_sb[:], bq_d.rearrange("(c p) -> p c", p=P))
            nc.sync.dma_start(bk_sb[:], bk_d.rearrange("(c p) -> p c", p=P))
            load_xb(0)
            nc.sync.dma_start(wv_sb[:], wv_d)
            nc.sync.dma_start(bv_row[:], bv_d.rearrange("(a m) -> a m", a=1))
            for g in range(1, NQB):
                load_x8(g)
                load_xb(g)
            nc.sync.dma_start(wo_sb[:], wo_d)

            # ---- constants ----
            ident_f = const.tile([P, P], F32)
            make_identity(nc, ident_f[:])
            ident = const.tile([P, P], BF)
            nc.vector.tensor_copy(ident[:], ident_f[:])

            # additive causal mask for diagonal blocks: tri[r, c] = 0 where
            # c >= r else -1000 (applied inside the scores PSUM accumulation)
            tri_f = const.tile([P, P], F32)
            nc.gpsimd.memset(tri_f[:], 0.0)
            nc.gpsimd.affine_select(
                out=tri_f[:], in_=tri_f[:],
                compare_op=mybir.AluOpType.is_ge, fill=-1000.0,
                base=0, channel_multiplier=-1, pattern=[[1, P]],
            )
            tri = const.tile([P, P], BF)
            nc.vector.tensor_copy(tri[:], tri_f[:])

            # bv broadcast rows [128, CD]
            bv_bc = const.tile([P, CD], F32)
            nc.gpsimd.partition_broadcast(bv_bc[:], bv_row[:])

            one_f = const.tile([P, 1], F32)
            nc.gpsimd.memset(one_f[:], 1.0)
            nc.vector.tensor_copy(
                v1[:, :, :, HD:HD + 1],
                one_f[:, 0:1].to_broadcast([P, ST, NH, 1]))

            def qk_chain(w_sb, dst, b_sb, cc, g, use_act=False):
                acc = ps.tile([P, QB], F32, tag="mm")
                for c in range(DRC):
                    nc.tensor.matmul(
                        acc[:],
                        w_sb[:, c, :, cc * P:(cc + 1) * P],
                        xt8_sb[:, c, :, g * QB:(g + 1) * QB],
                        start=(c == 0), stop=(c == DRC - 1),
                        perf_mode=DR)
                if use_act:
                    # boot only: ACT is idle pre-exp, so evacuate there and
                    # halve the serial DVE bias-add chain to the first scores
                    nc.scalar.activation(
                        dst[:, cc, g * QB:(g + 1) * QB], acc[:],
                        mybir.ActivationFunctionType.Identity,
                        bias=b_sb[:, cc:cc + 1])
                else:
                    nc.vector.tensor_scalar_add(
                        dst[:, cc, g * QB:(g + 1) * QB],
                        acc[:], b_sb[:, cc:cc + 1])

            def v_chain(tl, g):
                t = g * 4 + tl
                acc = ps.tile([P, QB], F32, tag="mm")
                vps = acc[:, :CD]
                for c in range(DC):
                    nc.tensor.matmul(
                        vps,
                        xtb_sb[:, c, t * P:(t + 1) * P],
                        wv_sb[:, c, :],
                        start=(c == 0), stop=(c == DC - 1))
                nc.vector.tensor_add(
                    v1[:, t, :, 0:HD],
                    vps.rearrange("p (h d) -> p h d", h=NH),
                    bv_bc.rearrange("p (h d) -> p h d", h=NH))

            def qk_atoms(g):
                """(cost_ns, thunk) atoms for block g q/k projections."""
                atoms = []
                for w_sb, dst, b_sb in ((wq_sb, q8, bq_sb), (wk_sb, k8, bk_sb)):
                    for cc in range(CC):
                        atoms.append((430, lambda w=w_sb, d=dst, b=b_sb,
                                      cc=cc: qk_chain(w, d, b, cc, g)))
                return atoms

            def v_atoms(g):
                """(deadline_j, cost_ns, thunk) atoms for block g v chains.
                v1[:, 4g+tl] is first read by the av matmul at step j=4g+tl
                of phase g, so each chain's emission deadline is that step."""
                return [(4 * g + tl, 860, lambda tl=tl: v_chain(tl, g))
                        for tl in range(4)]

            def make_d(g, hout, last=False):
                """Per-ql emitters for block g output (transpose, c_proj,
                store). Returns emit_ql(ql) -> list of (cost, thunk) atoms.
                The last block uses the then-idle sc/av PSUM tags so its
                tail chain is not gated by the mm tag ring."""
                ots = []
                for _i in range(2):
                    ot = outp.tile([P, 2, D], BF, tag="ot", bufs=2)
                    ots.append(ot)
                hts = {}

                def transp(ql):
                    # last block: ql>=2 rides the sc ring (idle once the
                    # final exp has been emitted), halving the per-ql ring
                    # serialization of the tail
                    tag = "sc" if (last and ql >= 2) else "mm"
                    pt2 = ps.tile([P, 2, P], BF, tag=tag)
                    for cc in range(CC):
                        nc.tensor.transpose(
                            pt2[:, cc, :],
                            hout[:, ql, cc * P:(cc + 1) * P],
                            ident[:])
                    hT = hpool.tile([P, CC, P], BF, tag="hT", bufs=4)
                    nc.vector.tensor_copy(hT[:], pt2[:])
                    hts[ql] = hT

                def po_piece(ql, nh_):
                    i, tl = ql // 2, ql % 2
                    tag = "sc" if (last and ql >= 2) else "mm"
                    pop = ps.tile([P, QB], F32, tag=tag)
                    for cc in range(CC):
                        nc.tensor.matmul(
                            pop[:],
                            hts[ql][:, cc, :],
                            wo_sb[:, cc, nh_ * QB:(nh_ + 1) * QB],
                            start=(cc == 0), stop=(cc == CC - 1))
                    # Pool/GPSIMD cannot read PSUM: evacuate on DVE; in the
                    # tail (post-final-exp) ACT takes half the copies
                    if last and ql >= 2:
                        nc.scalar.copy(
                            ots[i][:, tl, nh_ * QB:(nh_ + 1) * QB], pop[:])
                    else:
                        nc.vector.tensor_copy(
                            ots[i][:, tl, nh_ * QB:(nh_ + 1) * QB], pop[:])
                    if last:
                        # tail: store each half-row as soon as it is copied
                        nc.sync.dma_start(
                            out_v[:, g * 4 + ql, nh_ * QB:(nh_ + 1) * QB],
                            ots[i][:, tl, nh_ * QB:(nh_ + 1) * QB])
                    elif tl == 1 and nh_ == 1:
                        for half in range(2):
                            nc.sync.dma_start(
                                out_v[:, g * 4 + 2 * i + half, :],
                                ots[i][:, half, :])

                def emit_ql(ql):
                    return [(110, lambda: transp(ql)),
                            (430, lambda: po_piece(ql, 0)),
                            (430, lambda: po_piece(ql, 1))]
                return emit_ql

            class Fillers:
                """Budgeted interleaver: spends ACT slack on PE filler atoms.
                primary (next block's q/k projections) drains within the
                phase; mid (current block's v chains) carries a per-step
                deadline; secondary (c_proj of older blocks) is best-effort."""

                def __init__(self):
                    self.primary = []
                    self.mid = []        # (deadline_j, cost, thunk)
                    self.secondary = []
                    self.debt = 0.0

                def force_mid(self, j):
                    """Emit every mid atom whose deadline step has arrived."""
                    while self.mid and self.mid[0][0] <= j:
                        _, cost, thunk = self.mid.pop(0)
                        thunk()
                        self.debt -= cost

                def step(self, slack_ns):
                    self.debt += slack_ns
                    while True:
                        if self.primary:
                            cost, thunk = self.primary[0]
                            if cost > self.debt:
                                break
                            self.primary.pop(0)
                        elif self.mid:
                            _, cost, thunk = self.mid[0]
                            if cost > self.debt:
                                break
                            self.mid.pop(0)
                        elif self.secondary:
                            cost, thunk = self.secondary[0]
                            if cost > self.debt:
                                break
                            self.secondary.pop(0)
                        else:
                            break
                        thunk()
                        self.debt -= cost

                def drain(self, which):
                    lst = getattr(self, which)
                    while lst:
                        item = lst.pop(0)
                        item[-1]()
                    self.debt = 0.0

            MARGIN = 80.0

            def emit_scores(g, hp, j):
                """scores (+fused mask) for one step; returns (sc, L)."""
                m = j - 4 * g
                q0 = 128 * m if m > 0 else 0
                L = QB - q0
                sc = ps.tile([P, 2, QB], F32, tag="sc", bufs=2)
                for hh in range(2):
                    hb = 32 * (2 * hp + hh)
                    nc.tensor.matmul(
                        sc[:, hh, :L],
                        k8[hb:hb + 32, :, j * P:(j + 1) * P],
                        q8[hb:hb + 32, :, g * QB + q0:(g + 1) * QB],
                        start=True, stop=(m < 0), perf_mode=DR,
                        tile_position=(hb, 0))
                    if m >= 0:
                        # fused causal mask: sc[:, hh, :128] += tri
                        nc.tensor.matmul(
                            sc[:, hh, 0:P], ident[:], tri[:],
                            start=False, stop=True)
                return sc, L

            # PE warmup: junk transposes ramp the p-state to full clock
            # while the first DMAs are in flight
            warm = ps.tile([P, P], BF, tag="av")
            for _ in range(26):
                nc.tensor.transpose(warm[:], ident[:], ident[:])

            # pipeline: C(g) paced by ACT exp; q/k of g+1, v of g (by
            # deadline), and c_proj of older blocks fill PE slack
            fil = Fillers()
            for w_sb, dst, b_sb in ((wq_sb, q8, bq_sb), (wk_sb, k8, bk_sb)):
                for cc in range(CC):
                    qk_chain(w_sb, dst, b_sb, cc, 0, use_act=(dst is k8))
            fil.mid.extend(v_atoms(0))
            fil.primary.extend(qk_atoms(1))
            d_emit = {}

            # flat software-pipelined stream over (g, hp, j): scores for
            # step k+1 are emitted right after exp(k), ahead of step k's
            # av/division work, so exp never waits behind the step's own PE
            # work in the in-order queue
            steps = [(g, hp, j)
                     for g in range(NQB)
                     for hp in range(2)
                     for j in range(4 * g + 4)]
            state = {}
            houts = {}
            d_po = []

            def ensure_hp(g, hp):
                if (g, hp) in state:
                    return state[(g, hp)]
                if g not in houts:
                    hout = hpool.tile([P, 4, CD], BF, tag="hout", bufs=4)
                    houts[g] = hout
                ex = expp.tile([P, 2, ST, QB], BF, tag="ex")
                avs, rcps = [], []
                for _hh in range(2):
                    av = ps.tile([P, 4, HD + 1], F32, tag="av")
                    avs.append(av)
                    rcp = hpool.tile([P, 4], F32, tag="rcp", bufs=4)
                    rcps.append(rcp)
                st = dict(ex=ex, avs=avs, rcps=rcps)
                state[(g, hp)] = st
                return st

            sc, L = emit_scores(0, 0, 0)
            for k, (g, hp, j) in enumerate(steps):
                st = ensure_hp(g, hp)
                ex, avs, rcps = st["ex"], st["avs"], st["rcps"]
                m = j - 4 * g
                q0 = 128 * m if m > 0 else 0
                nc.scalar.activation(
                    ex[:, :, j, 0:L], sc[:, :, :L],
                    mybir.ActivationFunctionType.Exp,
                    scale=float(1.0 / np.sqrt(HD)))
                act_ns = 2 * L * ACT_NS + 217
                own = 0.0
                # next step's scores go out immediately behind this exp
                if k + 1 < len(steps):
                    g2, hp2, j2 = steps[k + 1]
                    if g2 != g:
                        fil.drain("primary")
                        if g2 < NQB - 1:
                            fil.primary.extend(qk_atoms(g2 + 1))
                        fil.mid.extend(v_atoms(g2))
                        d_emit[g] = make_d(g, houts[g])
                        fil.secondary.extend(
                            atom for ql in range(4)
                            for atom in d_emit[g](ql))
                    sc, L = emit_scores(g2, hp2, j2)
                    own += 2 * L * 0.21 + (214 if j2 >= 4 * g2 else 0)
                if m >= 0:
                    fil.force_mid(j)
                for hh in range(2):
                    for ql in range(max(m, 0), 4):
                        c0 = 128 * ql - q0
                        # start only once per av bank: the whole 2KB
                        # zero-region is marked pending, so the other ql
                        # chains zero-init on first touch
                        nc.tensor.matmul(
                            avs[hh][:, ql, :],
                            ex[:, hh, j, c0:c0 + P],
                            v1[:, j, 2 * hp + hh, :],
                            start=(j == 0 and ql == 0),
                            stop=(j == 4 * g + ql),
                            skip_group_check=True)
                        own += 28
                if m >= 0:
                    # av chain ql == m just stopped: divide inline
                    for hh in range(2):
                        h = 2 * hp + hh
                        nc.vector.reciprocal(
                            rcps[hh][:, m:m + 1], avs[hh][:, m:m + 1, HD])
                        nc.vector.tensor_scalar_mul(
                            houts[g][:, m, h * HD:(h + 1) * HD],
                            avs[hh][:, m, 0:HD], rcps[hh][:, m:m + 1])
                    if g == NQB - 1 and hp == 1:
                        # tail: transposes emit as each division lands; po
                        # atoms batch after ALL transposes so the DVE queue
                        # (hT copies) never sits behind ot copies
                        if g not in d_emit:
                            d_emit[g] = make_d(g, houts[g], last=True)
                        atoms = d_emit[g](m)
                        atoms[0][1]()
                        d_po.extend(atoms[1:])
                        if m == 3:
                            for _cost, thunk in d_po:
                                thunk()
                            d_po.clear()
                fil.step(act_ns - own - MARGIN)
            fil.drain("primary")
            fil.drain("secondary")

    nc.compile()
    return nc


def make_in_maps(x, w_attn, b_attn, w_proj):
    """Slice + pack full inputs into the 8 per-core input maps."""
    import ml_dtypes
    bf = ml_dtypes.bfloat16
    f8 = ml_dtypes.float8_e4m3fn
    x = np.asarray(x, dtype=np.float32)
    w_attn = np.asarray(w_attn, dtype=np.float32)
    b_attn = np.asarray(b_attn, dtype=np.float32)
    w_proj = np.asarray(w_proj, dtype=np.float32)

    def pack_qk(w):                      # [D, 256] -> lo/hi packed [D, 256]
        w4 = w.reshape(-1, NH, HD)
        return np.concatenate(
            [w4[:, :, :32].reshape(-1, P), w4[:, :, 32:].reshape(-1, P)], 1)

    def pack_qk_b(b):                    # [256] -> packed [256]
        b4 = b.reshape(NH, HD)
        return np.concatenate(
            [b4[:, :32].reshape(P), b4[:, 32:].reshape(P)], 0)

    def pack_dr_rows(a):                 # [D, M] -> [128, DRC, 2, M]
        return np.ascontiguousarray(
            a.reshape(DRC, 2, P, -1).transpose(2, 0, 1, 3))

    def pack_dc_rows(a):                 # [D, M] -> [128, DC, M]
        return np.ascontiguousarray(
            a.reshape(DC, P, -1).transpose(1, 0, 2))

    in_maps = []
    for cid in range(NCORES):
        b, hg = cid // GB, cid % GB
        cs = slice(hg * CD, (hg + 1) * CD)
        xT = np.ascontiguousarray(x[b].T)            # [D, S]
        in_maps.append({
            "xt8": pack_dr_rows(xT).astype(f8),
            "xtb": pack_dc_rows(xT).astype(bf),
            "wq": pack_dr_rows(pack_qk(w_attn[:, 0 * D:][:, cs])).astype(f8),
            "wk": pack_dr_rows(pack_qk(w_attn[:, 1 * D:][:, cs])).astype(f8),
            "wv": pack_dc_rows(w_attn[:, 2 * D:][:, cs]).astype(bf),
            "bq": np.ascontiguousarray(pack_qk_b(b_attn[0 * D:][cs])),
            "bk": np.ascontiguousarray(pack_qk_b(b_attn[1 * D:][cs])),
            "bv": np.ascontiguousarray(b_attn[2 * D:][cs]),
            "wo": np.ascontiguousarray(
                w_proj[hg * CD:(hg + 1) * CD, :].reshape(CC, P, D)
                .transpose(1, 0, 2)).astype(bf),
        })
    return in_maps


_RUN_KW = {}


def kernel(x, w_attn, b_attn, w_proj, b_proj):
    from concourse.bass_utils import run_bass_kernel_spmd

    nc = build_nc()
    in_maps = make_in_maps(x, w_attn, b_attn, w_proj)
    res = run_bass_kernel_spmd(nc, in_maps, core_ids=list(range(NCORES)),
                               **_RUN_KW)
    out = np.zeros((B, S, D), dtype=np.float32)
    for cid in range(NCORES):
        out[cid // GB] += np.asarray(res.results[cid]["out"],
                                     dtype=np.float32)
    out += np.asarray(b_proj, dtype=np.float32)
    globals()["_LAST_RESULTS"] = res
    return out
